# revision 35
# baseline (speedup 1.0000x reference)
"""GroupNorm + 4-head self-attention + output projection, TRN2 Bass kernel.

Sharding: 8 cores = 4 batches x 2 query-halves.  Each core runs GroupNorm and
the full K/V projection for its batch (duplicated across the 2 cores of a
batch, ~5% extra FLOPs) and attention + output projection for its 2048-query
chunk.  The query chunk is rotated to the front of the token axis on the host
(GroupNorm stats / K / V are permutation-invariant along tokens), so all 8
cores run one identical SPMD program and the unshard is pure concatenation.

Device layout (per core).  The kernel is softmax-throughput bound: the exp of
the 4096x2048x4 sim matrix is the largest single cost, so it is split across
two engines per i-chunk:
  heads 0,1: ACT exp (exact, table-based), bf16 out
  heads 2,3: DVE Schraudolph fast-exp -- one tensor_scalar computing
             round(sim * SCALE*log2e*128 + (127-c)*128) into a uint16 tile,
             whose bits reinterpreted as bf16 equal exp(SCALE*sim) within
             +-3%; fp32->uint16 conversion saturates at 0 (underflow -> +0.0)
             and rounds half-even.  The attn@V / denominator matmuls read the
             tile through .bitcast(bf16).
The softmax denominator normalizes per-head, so the fast-exp's systematic
component cancels; end-to-end rel err ~6e-3 (gate 2e-2).

Pipeline per (j, i): 4 sim matmuls (row-tiled 4-up, concurrent), ACT exp pr0
+ DVE fexp pr1 in parallel, then av/dn matmuls of the PREVIOUS i (col-tiled
4-up) so the PE FIFO never waits on exp.  PSUM: sim ring 3 slots x 2 banks +
oacc 1 + dn 1 = 8 banks.  GroupNorm fp32 (bn_stats -> group merge via tiny
block-ones matmul -> rstd via Ln/Exp -> broadcast), xn written as bf16;
QKV / output projections in bf16 (weights pre-converted host-side).
"""

import numpy as np

HEAD = 4
DIM_HEAD = 32
DIM = 256
GROUPS = 32
EPS = 1e-5
SCALE = DIM_HEAD ** -0.5
N = 4096
NQ = 2048
NCORES = 8
P = 128
JW = 512           # query-chunk width per inner tile
NJ = NQ // JW      # 4
NI = N // P        # 32 key chunks

LOG2E = 1.4426950408889634
FE_A = float(SCALE * LOG2E * 128.0)      # fast-exp multiplier (scale folded)
FE_B = float((127.0 - 0.0430) * 128.0)   # fast-exp bias (Schraudolph c)

_cache = {}


def _get_nc():
    if "nc" in _cache:
        return _cache["nc"]
    from contextlib import ExitStack

    import concourse.bass as bass  # noqa: F401
    import concourse.tile as tile
    from concourse import bacc, mybir

    f32 = mybir.dt.float32
    b16 = mybir.dt.bfloat16
    u16 = mybir.dt.uint16
    AF = mybir.ActivationFunctionType
    ALU = mybir.AluOpType

    # Confine Exp/Ln to the one table set that holds both, so the table-load
    # pass never alternates sets (each switch costs ~1.3us of ACT time).
    # Membership-only edit: set order (= act_func_set_id) is preserved.
    import concourse.bacc as bacc_mod
    from concourse.hw_specs import get_activation_tables as _orig_tables

    def _tables_one_exp_ln_set(arch):
        combo = "natural_log_exp_and_others"
        out = {}
        for name, fns in _orig_tables(arch).items():
            if name != combo:
                fns = {f for f in fns if f not in (AF.Exp, AF.Ln)}
            out[name] = fns
        return out

    bacc_mod.get_activation_tables = _tables_one_exp_ln_set

    nc = bacc.Bacc(None, target_bir_lowering=False)
    x_in = nc.declare_dram_parameter("x", [DIM, N], b16, isOutput=False)
    wqkvT = nc.declare_dram_parameter("wqkvT", [DIM, 3 * P], b16, isOutput=False)
    woutT = nc.declare_dram_parameter("woutT", [P, DIM], b16, isOutput=False)
    # small fp32 constants packed into one tensor / one DMA:
    # cols 0-1 gnw(t0,t1), 2-3 gnb, 4-5 bout, 6-21 blk8
    misc = nc.declare_dram_parameter("misc", [P, 22], f32, isOutput=False)
    blk8T = nc.declare_dram_parameter("blk8T", [16, P], f32, isOutput=False)
    e4 = nc.declare_dram_parameter("e4", [P, P], b16, isOutput=False)
    y_out = nc.declare_dram_parameter("y", [DIM, NQ], f32, isOutput=True)

    with ExitStack() as ctx:
        tc = ctx.enter_context(tile.TileContext(nc))
        const = ctx.enter_context(tc.tile_pool(name="const", bufs=1))
        persist = ctx.enter_context(tc.tile_pool(name="persist", bufs=1))
        work = ctx.enter_context(tc.tile_pool(name="work", bufs=3))
        attnp = ctx.enter_context(tc.tile_pool(name="attnp", bufs=2))
        # PSUM budget (8 banks): sim ring 3 slots x 2 banks + oacc 1 + dn 1
        psA = ctx.enter_context(tc.tile_pool(name="psA", bufs=3, space="PSUM"))
        psB = ctx.enter_context(tc.tile_pool(name="psB", bufs=1, space="PSUM"))

        # ---- DMA order (one sync queue, issue-rate-bound): x t0 chunks
        # first (GroupNorm stats chase them), then the small consts needed
        # by the t0 chain, then x t1, then the projection weights.
        xb = [persist.tile([P, N], b16, tag=f"xb{t}", name=f"xb{t}")
              for t in range(2)]
        for ch in range(8):
            nc.sync.dma_start(
                out=xb[0][:, ch * 512:(ch + 1) * 512],
                in_=x_in[0:P, ch * 512:(ch + 1) * 512],
            )
        misc_sb = const.tile([P, 22], f32, tag="misc")
        nc.sync.dma_start(out=misc_sb, in_=misc[:, :])
        gnw_sb = [misc_sb[:, t:t + 1] for t in range(2)]
        gnb_sb = [misc_sb[:, 2 + t:3 + t] for t in range(2)]
        bout_sb = [misc_sb[:, 4 + t:5 + t] for t in range(2)]
        blk8_sb = misc_sb[:, 6:22]
        blk8T_sb = const.tile([16, P], f32, tag="blk8T")
        nc.sync.dma_start(out=blk8T_sb, in_=blk8T[:, :])
        for ch in range(8):
            nc.sync.dma_start(
                out=xb[1][:, ch * 512:(ch + 1) * 512],
                in_=x_in[P:2 * P, ch * 512:(ch + 1) * 512],
            )
        wqkv_sb = []
        for t in range(2):
            w = const.tile([P, 3 * P], b16, tag=f"wqkv{t}", name=f"wqkv{t}")
            nc.sync.dma_start(out=w, in_=wqkvT[t * P:(t + 1) * P, :])
            wqkv_sb.append(w)
        wout_sb = const.tile([P, DIM], b16, tag="wout")
        nc.sync.dma_start(out=wout_sb, in_=woutT[:, :])
        e4_sb = const.tile([P, P], b16, tag="e4")
        nc.sync.dma_start(out=e4_sb, in_=e4[:, :])
        dnc = const.tile([P, JW], b16, tag="dnc")
        nc.vector.memset(dnc, 1.0)
        ones_sb = const.tile([P, 1], b16, tag="ones")
        nc.vector.memset(ones_sb, 1.0)
        eps_sb = const.tile([16, 1], f32, tag="eps")
        nc.vector.memset(eps_sb, EPS)

        # Pre-fill the (single) dn psum slot so partitions the denominator
        # matmuls never write hold finite values: lets the epilogue stage all
        # 4 denom rows with ONE [97, 512] copy (cost is free-dim-bound).
        dnfill = psB.tile([P, JW], f32, tag="dn", name="dnfill")
        nc.vector.memset(dnfill, 1.0)

        wqs = [persist.tile([P, 3 * P], b16, tag=f"wqs{t}", name=f"wqs{t}")
               for t in range(2)]
        be16 = [persist.tile([P, 1], b16, tag=f"be16{t}", name=f"be16{t}")
                for t in range(2)]
        qkvb_ps = psA.tile([P, 4], f32, tag="sim")

        # ---------------- GroupNorm ----------------
        # Per-tile pipeline in DVE-FIFO execution order: t0's stats arrive
        # first (x DMA order), so run t0's full chain and its bf16 conversion
        # in the shadow of t1's x DMA, then t1's chain.
        albe = []
        for t in range(2):
            stats = work.tile([P, 8, 6], f32, tag=f"stats{t}", name=f"stats{t}")
            for ch in range(8):
                nc.vector.bn_stats(
                    out=stats[:, ch, :], in_=xb[t][:, ch * 512:(ch + 1) * 512]
                )
            mv = work.tile([P, 2], f32, tag=f"mv{t}", name=f"mv{t}")
            nc.vector.bn_aggr(out=mv, in_=stats)
            # mv col1 := var + mean^2  (= E[x^2] per channel)
            msq = work.tile([P, 1], f32, tag=f"msq{t}", name=f"msq{t}")
            nc.vector.tensor_mul(msq, mv[:, 0:1], mv[:, 0:1])
            nc.vector.tensor_add(mv[:, 1:2], mv[:, 1:2], msq)
            # per-group (mean, E[x^2]) via block-ones (1/8) matmul
            gst_ps = psB.tile([16, 2], f32, tag="dn", name=f"gst_ps{t}")
            nc.tensor.matmul(gst_ps, lhsT=blk8_sb, rhs=mv, start=True, stop=True)
            gst = work.tile([16, 2], f32, tag=f"gst{t}", name=f"gst{t}")
            nc.vector.tensor_copy(gst, gst_ps)
            mmg = work.tile([16, 1], f32, tag=f"mmg{t}", name=f"mmg{t}")
            nc.vector.tensor_mul(mmg, gst[:, 0:1], gst[:, 0:1])
            varg = work.tile([16, 1], f32, tag=f"varg{t}", name=f"varg{t}")
            nc.vector.tensor_sub(varg, gst[:, 1:2], mmg)
            # rstd = exp(-0.5*ln(var+eps)): ln+exp share one ACT table set
            # with the attention exps (no extra ~2.7us table reload)
            sdg = work.tile([16, 1], f32, tag=f"sdg{t}", name=f"sdg{t}")
            nc.scalar.activation(
                out=sdg, in_=varg, func=AF.Ln, bias=eps_sb, scale=1.0
            )
            ms = work.tile([16, 2], f32, tag=f"ms{t}", name=f"ms{t}")
            nc.vector.tensor_copy(ms[:, 0:1], gst[:, 0:1])
            nc.scalar.activation(
                out=ms[:, 1:2], in_=sdg, func=AF.Exp, scale=-0.5
            )
            # broadcast group (mean, rstd) to the 8 channels of each group
            cb_ps = psB.tile([P, 2], f32, tag="oacc", name=f"cb_ps{t}")
            nc.tensor.matmul(cb_ps, lhsT=blk8T_sb, rhs=ms,
                             start=True, stop=True)
            al = persist.tile([P, 1], f32, tag=f"alpha{t}", name=f"alpha{t}")
            nc.vector.tensor_mul(al, cb_ps[:, 1:2], gnw_sb[t])
            tmpb = work.tile([P, 1], f32, tag=f"tmpb{t}", name=f"tmpb{t}")
            nc.vector.tensor_mul(tmpb, cb_ps[:, 0:1], al)
            be = persist.tile([P, 1], f32, tag=f"beta{t}", name=f"beta{t}")
            nc.vector.tensor_sub(be, gnb_sb[t], tmpb)
            albe.append((al, be))
        # ---- fold GroupNorm into the projections: q = Wq'(x_bf) + qb,
        # Wq' = Wq diag(alpha), qb = Wq beta (same for k); the V bias
        # telescopes through attention (sum_m attn*vb = vb*denominator)
        # into the output projection bias: bout2 = bout + Wout vb.
        for t in range(2):
            al, be = albe[t]
            nc.vector.tensor_scalar(out=wqs[t], in0=wqkv_sb[t], scalar1=al,
                                    scalar2=None, op0=ALU.mult)
            nc.vector.tensor_copy(be16[t], be)
        for sel in range(3):
            for t in range(2):
                nc.tensor.matmul(
                    qkvb_ps[:, sel:sel + 1],
                    lhsT=wqkv_sb[t][:, sel * P:(sel + 1) * P],
                    rhs=be16[t], start=(t == 0), stop=(t == 1),
                )
        qb = persist.tile([P, 1], f32, tag="qb")
        nc.vector.tensor_copy(qb, qkvb_ps[:, 0:1])
        kb = persist.tile([P, 1], f32, tag="kb")
        nc.vector.tensor_copy(kb, qkvb_ps[:, 1:2])
        vb16 = persist.tile([P, 1], b16, tag="vb16")
        nc.vector.tensor_copy(vb16, qkvb_ps[:, 2:3])
        bout2 = []
        for t in range(2):
            bo_ps = psA.tile([P, 1], f32, tag="sim", name=f"bo_ps{t}")
            nc.tensor.matmul(bo_ps, lhsT=wout_sb[:, t * P:(t + 1) * P],
                             rhs=vb16, start=True, stop=True)
            bo2 = persist.tile([P, 1], f32, tag=f"bo2{t}", name=f"bo2{t}")
            nc.vector.tensor_add(bo2, bo_ps, bout_sb[t])
            bout2.append(bo2)

        # ---------------- QKV projections ----------------
        qT = persist.tile([P, NQ], b16, tag="qT")
        kT = persist.tile([P, N], b16, tag="kT")
        vS = persist.tile([P, N], b16, tag="vS")   # vS[p, i*128+o] = v[i*128+p, o]

        def emit_q(jq):
            ps = psA.tile([P, 2, JW], f32, tag="sim")
            for t in range(2):
                nc.tensor.matmul(
                    ps[:, 0, :], lhsT=wqs[t][:, 0:P],
                    rhs=xb[t][:, jq * 512:(jq + 1) * 512],
                    start=(t == 0), stop=(t == 1),
                )
            nc.vector.tensor_scalar(out=qT[:, jq * 512:(jq + 1) * 512],
                                    in0=ps[:, 0, :], scalar1=qb,
                                    scalar2=None, op0=ALU.add)

        def emit_k(jk):
            ps = psA.tile([P, 2, JW], f32, tag="sim")
            for t in range(2):
                nc.tensor.matmul(
                    ps[:, 0, :], lhsT=wqs[t][:, P:2 * P],
                    rhs=xb[t][:, jk * 512:(jk + 1) * 512],
                    start=(t == 0), stop=(t == 1),
                )
            nc.scalar.activation(out=kT[:, jk * 512:(jk + 1) * 512],
                                 in_=ps[:, 0, :], func=AF.Identity,
                                 bias=kb, scale=1.0)

        vT = persist.tile([P, N], b16, tag="vT")  # v in [o, m] (w-stationary)

        def emit_vT(ch):
            # one 512-token chunk of v, produced weight-stationary (2 wide
            # matmuls instead of 8 narrow ones), then rotated into the
            # attention layout vS[m, o] by 4 hardware DMA transposes.
            ps = psA.tile([P, 2, JW], f32, tag="sim", name="vtps")
            for t in range(2):
                nc.tensor.matmul(
                    ps[:, 0, :], lhsT=wqs[t][:, 2 * P:3 * P],
                    rhs=xb[t][:, ch * 512:(ch + 1) * 512],
                    start=(t == 0), stop=(t == 1),
                )
            if ch % 2 == 0:
                nc.scalar.activation(out=vT[:, ch * 512:(ch + 1) * 512],
                                     in_=ps[:, 0, :], func=AF.Copy)
            else:
                nc.vector.tensor_copy(vT[:, ch * 512:(ch + 1) * 512],
                                      ps[:, 0, :])
            for blk in range(4 * ch, 4 * ch + 4):
                nc.sync.dma_start_transpose(
                    out=vS[:, blk * P:(blk + 1) * P],
                    in_=vT[:, blk * P:(blk + 1) * P],
                )

        # Produce only what attention j=0 needs up front; the rest (q 1-3,
        # k 1-7, v 4-31) is emitted interleaved into j=0's i-loop so the
        # first exp starts early.
        emit_q(0)
        emit_k(0)
        emit_vT(0)

        # ---------------- attention ----------------
        # Per-j epilogue is emitted as 5 pieces interleaved into the first
        # iterations of the NEXT j (overlaps its serial chain with compute
        # and keeps the PE warm across the boundary).
        def make_epilogue(j, oacc, dn):
            def p0():
                # stage all 4 denom rows (psum partitions 0/32/64/96) into
                # dnc with one copy; partitions in between carry finite
                # psum garbage that e4's zero weights annihilate.  Then
                # select+broadcast each head's row to its 32 hidden
                # partitions and take ln (rcb = exp(-ln d) = 1/d on ACT,
                # avoiding the slow DVE iterative-divide reciprocal).
                nc.scalar.activation(out=dnc[0:97, :], in_=dn[0:97, :],
                                     func=AF.Copy)
                dbc_ps = psA.tile([P, JW], f32, tag="sim")
                nc.tensor.matmul(
                    dbc_ps, lhsT=e4_sb, rhs=dnc, start=True, stop=True
                )
                lnd = work.tile([P, JW], f32, tag="lnd")
                nc.scalar.activation(out=lnd, in_=dbc_ps, func=AF.Ln)
                return lnd

            def p1(lnd):
                rcb = work.tile([P, JW], f32, tag="rcb")
                nc.scalar.activation(out=rcb, in_=lnd, func=AF.Exp, scale=-1.0)
                ao = work.tile([P, JW], b16, tag="ao")
                nc.vector.tensor_mul(ao, oacc, rcb)
                return ao

            def p2(ao, t):
                yps = psA.tile([P, JW], f32, tag="sim")
                nc.tensor.matmul(
                    yps, lhsT=wout_sb[:, t * P:(t + 1) * P], rhs=ao,
                    start=True, stop=True,
                )
                ysb = work.tile([P, JW], f32, tag="ysb")
                nc.vector.tensor_scalar_add(ysb, yps, bout2[t])
                nc.sync.dma_start(
                    out=y_out[t * P:(t + 1) * P, j * JW:(j + 1) * JW], in_=ysb
                )

            state = {}

            def run_piece(k):
                if k == 0:
                    state["lnd"] = p0()
                elif k == 1:
                    state["ao"] = p1(state["lnd"])
                elif k == 2:
                    p2(state["ao"], 0)
                elif k == 3:
                    p2(state["ao"], 1)

            return run_piece

        AVDELAY = 3
        epilogue = None
        pending = []        # av/dn emission pipeline, carried ACROSS j
        for j in range(NJ):
            oacc = psB.tile([P, JW], f32, tag="oacc")
            dn = psB.tile([P, JW], f32, tag="dn")

            def emit_avdn(i, at0, at1, oacc=oacc, dn=dn):
                rhss = [at0[:, 0, :], at0[:, 1, :],
                        at1[:, 0, :].bitcast(b16), at1[:, 1, :].bitcast(b16)]
                for h in range(HEAD):
                    nc.tensor.matmul(
                        oacc[32 * h:32 * h + 32, :],
                        lhsT=vS[:, i * P + 32 * h:i * P + 32 * h + 32],
                        rhs=rhss[h],
                        start=(i == 0), stop=(i == NI - 1),
                        tile_position=(0, 32 * h),
                        skip_group_check=True,
                    )
                for h in range(HEAD):
                    nc.tensor.matmul(
                        dn[32 * h:32 * h + 1, :],
                        lhsT=ones_sb,
                        rhs=rhss[h],
                        start=(i == 0), stop=(i == NI - 1),
                        tile_position=(0, 32 * h),
                        skip_group_check=True,
                    )

            for i in range(NI):
                if j == 0:
                    la = i + 4          # lookahead production for chunk i+4
                    if la < NI and la % 4 == 0:
                        emit_k(la // 4)
                        emit_vT(la // 4)
                    if i in (1, 3, 5):
                        emit_q(1 + i // 2)
                sims = []
                for pr in range(2):
                    sim = psA.tile([P, 2, JW], f32, tag="sim")
                    for hh in range(2):
                        h = pr * 2 + hh
                        nc.tensor.matmul(
                            sim[:, hh, :],
                            lhsT=kT[32 * h:32 * h + 32, i * P:(i + 1) * P],
                            rhs=qT[32 * h:32 * h + 32, j * JW:(j + 1) * JW],
                            start=True, stop=True,
                            tile_position=(32 * h, 0),
                        )
                    sims.append(sim)
                # heads 0,1: exact exp on ACT; heads 2,3: fast-exp on DVE
                at0 = attnp.tile([P, 2, JW], b16, tag="at0", bufs=5)
                nc.scalar.activation(out=at0, in_=sims[0], func=AF.Exp,
                                     scale=SCALE)
                at1 = attnp.tile([P, 2, JW], u16, tag="at1", bufs=5)
                nc.vector.tensor_scalar(
                    out=at1, in0=sims[1], scalar1=FE_A, scalar2=FE_B,
                    op0=ALU.mult, op1=ALU.add,
                )
                pending.append((emit_avdn, i, at0, at1))
                if len(pending) > AVDELAY:
                    fn, ii, a0, a1 = pending.pop(0)
                    fn(ii, a0, a1)
                if epilogue is not None and 2 <= i < 6:
                    epilogue(i - 2)
                    if i == 5:
                        epilogue = None
            epilogue = make_epilogue(j, oacc, dn)
        for fn, ii, a0, a1 in pending:
            fn(ii, a0, a1)
        for k in range(4):
            epilogue(k)

    nc.finalize()
    _cache["nc"] = nc
    return nc


def _prep_in_maps(x, gn_weight, gn_bias, w_qkv, w_out, b_out):
    import ml_dtypes

    f = np.float32
    bf = ml_dtypes.bfloat16
    x = np.asarray(x, dtype=f).astype(bf)
    wqkvT = np.ascontiguousarray(np.asarray(w_qkv, dtype=f).T.astype(bf))
    woutT = np.ascontiguousarray(np.asarray(w_out, dtype=f).T.astype(bf))
    gnw = np.asarray(gn_weight, dtype=f).reshape(2, P)
    gnb = np.asarray(gn_bias, dtype=f).reshape(2, P)
    bo = np.asarray(b_out, dtype=f).reshape(2, P)
    ar = np.arange(P)
    # misc pack: cols 0-1 gnw(t0,t1), 2-3 gnb, 4-5 bout, 6-21 blk8
    misc = np.zeros((P, 22), f)
    misc[:, 0] = gnw[0]
    misc[:, 1] = gnw[1]
    misc[:, 2] = gnb[0]
    misc[:, 3] = gnb[1]
    misc[:, 4] = bo[0]
    misc[:, 5] = bo[1]
    misc[ar, 6 + ar // 8] = 0.125
    blk8T = np.zeros((16, P), f)
    blk8T[ar // 8, ar] = 1.0
    # selector/broadcast: out[q] = in[32*(q//32)] — picks each head's denom
    # row (at partition 32h) and fans it out to that head's 32 partitions
    e4 = np.zeros((P, P), ml_dtypes.bfloat16)
    e4[32 * (ar // 32), ar] = 1.0
    shared = dict(wqkvT=wqkvT, woutT=woutT, misc=misc, blk8T=blk8T, e4=e4)
    in_maps = []
    for core in range(NCORES):
        b, half = divmod(core, 2)
        xb = x[b].reshape(DIM, N)
        if half == 0:
            xp = np.ascontiguousarray(xb)
        else:
            xp = np.ascontiguousarray(
                np.concatenate([xb[:, NQ:], xb[:, :NQ]], axis=1)
            )
        in_maps.append(dict(x=xp, **shared))
    return in_maps


def _get_executor():
    """Build the sharded jitted executor once (compiles the NEFF once).

    Returns (exec_fn, meta): exec_fn takes a list of 8 per-core input dicts
    and returns the list of 8 per-core output dicts.  Mirrors
    concourse.bass2jax.run_bass_via_pjrt's multi-core path but caches the
    jax.jit so repeated calls don't recompile.
    """
    if "exec" in _cache:
        return _cache["exec"]
    import jax
    import concourse.mybir as mybir
    from jax.sharding import Mesh, PartitionSpec
    from jax.experimental.shard_map import shard_map
    from concourse import bass2jax

    bass2jax.install_neuronx_cc_hook()
    nc = _get_nc()

    partition_name = (
        nc.partition_id_tensor.name if nc.partition_id_tensor else None
    )
    in_names, out_names, out_avals, zero_outs = [], [], [], []
    for alloc in nc.m.functions[0].allocations:
        if not isinstance(alloc, mybir.MemoryLocationSet):
            continue
        name = alloc.memorylocations[0].name
        if alloc.kind == "ExternalInput":
            if name != partition_name:
                in_names.append(name)
        elif alloc.kind == "ExternalOutput":
            shape = tuple(alloc.tensor_shape)
            dtype = mybir.dt.np(alloc.dtype)
            out_names.append(name)
            out_avals.append(jax.core.ShapedArray(shape, dtype))
            zero_outs.append(np.zeros(shape, dtype))
    n_params = len(in_names)
    n_outs = len(out_names)
    all_names = in_names + out_names
    if partition_name is not None:
        all_names = all_names + [partition_name]

    def _body(*args):
        operands = list(args)
        if partition_name is not None:
            operands.append(bass2jax.partition_id_tensor())
        outs = bass2jax._bass_exec_p.bind(
            *operands,
            out_avals=tuple(out_avals),
            in_names=tuple(all_names),
            out_names=tuple(out_names),
            lowering_input_output_aliases=(),
            sim_require_finite=True,
            sim_require_nnan=True,
            nc=nc,
        )
        return tuple(outs)

    devices = jax.devices()[:NCORES]
    mesh = Mesh(np.asarray(devices), ("core",))
    sharded = jax.jit(
        shard_map(
            _body, mesh=mesh,
            in_specs=(PartitionSpec("core"),) * (n_params + n_outs),
            out_specs=(PartitionSpec("core"),) * n_outs,
            check_rep=False,
        ),
        keep_unused=True,
    )
    from jax.sharding import NamedSharding
    sharding = NamedSharding(mesh, PartitionSpec("core"))
    dev_zeros = [
        jax.device_put(
            np.zeros((NCORES * z.shape[0], *z.shape[1:]), z.dtype), sharding
        )
        for z in zero_outs
    ]

    def put_inputs(in_maps):
        return [
            jax.device_put(
                np.concatenate([np.asarray(m[name]) for m in in_maps], axis=0),
                sharding,
            )
            for name in in_names
        ]

    def run_device(device_inputs):
        return sharded(*device_inputs, *dev_zeros)

    def exec_fn(in_maps, device_inputs=None):
        if device_inputs is None:
            device_inputs = put_inputs(in_maps)
        out_arrs = [np.asarray(a) for a in run_device(device_inputs)]
        return [
            {
                name: out_arrs[i].reshape(NCORES, *out_avals[i].shape)[c]
                for i, name in enumerate(out_names)
            }
            for c in range(NCORES)
        ]

    meta = dict(in_names=in_names, out_names=out_names, mesh=mesh,
                sharded=sharded, zero_outs=zero_outs,
                put_inputs=put_inputs, run_device=run_device)
    _cache["exec"] = (exec_fn, meta)
    return _cache["exec"]


def _assemble(results):
    y = np.empty((4, DIM, N), np.float32)
    for core in range(NCORES):
        b, half = divmod(core, 2)
        y[b][:, half * NQ:(half + 1) * NQ] = results[core]["y"]
    return y.reshape(4, DIM, 64, 64)


def _run(inputs, **kw):
    exec_fn, _ = _get_executor()
    in_maps = _prep_in_maps(**inputs)
    results = exec_fn(in_maps)
    return _assemble(results), results


def kernel(x, gn_weight, gn_bias, w_qkv, w_out, b_out):
    out, _ = _run(dict(x=x, gn_weight=gn_weight, gn_bias=gn_bias,
                       w_qkv=w_qkv, w_out=w_out, b_out=b_out))
    return out
